# revision 24
# baseline (speedup 1.0000x reference)
"""Trainium2 Bass kernel for the stacked-Chebyshev locally-connected net.

Reference computation (B=256, k=6250, d*d=4096, O=10):
    x1 = z @ (mask*T1).T
    x2 = 2*(z @ (mask*T2).T)*x1 - T0
    x3 = 2*(z @ (mask*T3).T)*x2 - x1
    out = x3 @ C_w.T + C_b

The mask is a locally-connected conv pattern: 16x16 patch, stride 2, 25x25
positions, stacked 10x.  Rows sharing patch-row index i have a contiguous
1024-wide support in d (8 chunks of 2 image rows); grouping by i cuts the
contraction from 4096 to 1024.  Sorting each group's 250 k-columns by
patch-column j and splitting 125/125 narrows the support inside every
chunk to 40 of 64 image columns = 80 of 128 rows (ascending column
interleave for the low-j "A" tile, descending for the high-j "B" tile).
The 8 per-chunk 80-row windows are CONCATENATED into one 640-row
contraction stream = exactly 5 full K=128 matmuls instead of 8 — 37.5%
fewer PE cycles and 37.5% less weight DMA than the dense form, with all
transfers spanning 128 partitions (full SDMA port utilization) and all
matmuls full-K (the HAM clock-gate only stays at 2.4 GHz under full-row
activity).  The price: the z stream becomes per-(slot, tile) instead of
shared, which is small next to the weight savings.

Sharding: 25 i-groups over 8 cores; every core gets 3 whole groups plus
1/8 of group 24 (a 32-column "mini" unit, j-sorted per core so its
support fits a host-shifted 48-row window; 8 x 48 = 384 = 3 matmuls).

Performance structure:
  - the two HWDGE rings carry (z, w) of each virtual slot on opposite
    rings, in exact PE consumption order; ~16 large DMAs total.
  - dummy matmuls on a memset tile run during the DMA preamble so the
    PE's HAM clock-gate is warm when the real matmuls start.
  - all matmuls are fp16 (fast-weight-load stays on, LDWEIGHTS hides).
  - the Chebyshev recurrence is split across the scalar (ACT) and vector
    (DVE) engines; per-core partial outputs are summed on the host.
"""

import numpy as np

import concourse.bass as bass
import concourse.mybir as mybir
import concourse.tile as tile
from concourse import bacc
from concourse.bass_utils import run_bass_kernel_spmd

F32 = mybir.dt.float32
F16 = mybir.dt.float16

B = 256          # batch
O = 10           # output classes
D2 = 4096        # d*d
N_CORES = 8
FULL_SLOTS = 3   # whole groups per core
WC = 80          # window rows per chunk (40 image cols x 2 rows)
NK = 5           # K=128 matmuls per (slot, tile, layer): 8*WC/128
WM = 48          # mini window rows per chunk
NKM = 3          # K=128 matmuls per mini layer: 8*WM/128
MINI_COLS = 32   # k-columns of the shared group-24 mini unit (<=32 real)
G_SPLIT = 24     # the group split across all 8 cores
N_UNITS = 2 * FULL_SLOTS + 1
DUMMY_MMS = 14   # HAM warm-up matmuls during the DMA preamble

# columns of group G_SPLIT owned by each core (6x31 + 2x32 = 250)
_MINI_N = (31, 31, 31, 31, 31, 31, 32, 32)
_MINI_OFF = tuple(int(x) for x in np.cumsum((0,) + _MINI_N[:-1]))


def _group_cols_jmajor(i):
    """k-column indices of group i, j-major (j outer, stack inner)."""
    return np.array(
        [s * 625 + i * 25 + j for j in range(25) for s in range(10)], dtype=np.int64
    )


# window pixel indices within a 128-px chunk (2 image rows of 64):
# ascending layout index w <-> pixel (w%2)*64 + w//2     (cols 0..39)
# descending layout index w <-> pixel (w%2)*64 + 63-w//2 (cols 63..24)
_WIN = (np.array([(w % 2) * 64 + w // 2 for w in range(WC)], dtype=np.int64),
        np.array([(w % 2) * 64 + 63 - w // 2 for w in range(WC)], dtype=np.int64))


def _build_nc():
    nc = bacc.Bacc(
        "TRN2", target_bir_lowering=False, debug=False, num_devices=N_CORES
    )
    # z contraction streams: row block (t*3+s) holds the 640-row concat
    # chunk-blocked as [128, NK*B]
    zst = nc.dram_tensor("zst", [6 * 128, NK * B], F16, kind="ExternalInput").ap()
    # weights: row block (t*3+s), cols (li*NK + k)*128 + m
    wall = nc.dram_tensor("wall", [6 * 128, 3 * NK * 128], F16,
                          kind="ExternalInput").ap()
    # mini z stream and weights
    zgm = nc.dram_tensor("zgm", [128, NKM * B], F16, kind="ExternalInput").ap()
    wm = nc.dram_tensor("wm", [128, 3 * NKM * MINI_COLS], F16,
                        kind="ExternalInput").ap()
    # negated T0 (additive bias on the scalar engine); col = unit index
    t0n = nc.dram_tensor("t0n", [128, 8], F32, kind="ExternalInput").ap()
    cwt = nc.dram_tensor("cwt", [128, N_UNITS * O], F16, kind="ExternalInput").ap()
    out = nc.dram_tensor("out", [O, B], F32, kind="ExternalOutput").ap()

    with tile.TileContext(nc) as tc:
        with (
            tc.tile_pool(name="zpool", bufs=1) as zpool,
            tc.tile_pool(name="cpool", bufs=1) as cpool,
            tc.tile_pool(name="wpool", bufs=1) as wpool,
            tc.tile_pool(name="xpool", bufs=6) as xpool,
            tc.tile_pool(name="tpool", bufs=4) as tpool,
            tc.tile_pool(name="ppool", bufs=7, space="PSUM") as ppool,
            tc.tile_pool(name="opool", bufs=1, space="PSUM") as opool,
        ):
            dz = zpool.tile([128, B], F16, tag="dz")
            nc.vector.memset(dz[:], 0.0)

            zt, wt = {}, {}
            for t in range(2):
                for s in range(FULL_SLOTS):
                    ztile = zpool.tile([128, NK * B], F16, tag=f"z{s}{t}",
                                       name=f"z{s}{t}")
                    wtile = wpool.tile([128, 3 * NK * 128], F16,
                                       tag=f"w{s}{t}", name=f"w{s}{t}")
                    zt[(s, t)] = ztile
                    wt[(s, t)] = wtile
            zgt = cpool.tile([128, NKM * B], F16, tag="zg")
            wm_sb = cpool.tile([128, 3 * NKM * MINI_COLS], F16, tag="wm")
            t0_sb = cpool.tile([128, 8], F32, tag="t0")
            cw_sb = cpool.tile([128, N_UNITS * O], F16, tag="cw")

            def vs_dma(zeng, s, t):
                r = (t * 3 + s) * 128
                zeng.dma_start(zt[(s, t)][:], zst[r:r + 128, :])
                weng = nc.scalar if zeng is nc.sync else nc.sync
                weng.dma_start(wt[(s, t)][:], wall[r:r + 128, :])

            # ---- DMA issue plan: (z, w) of each vslot on opposite rings,
            # in consumption order; the very first vslot's weights arrive
            # layer-1 first so matmuls can start early ----
            r0 = 0
            nc.sync.dma_start(zt[(0, 0)][:], zst[r0:r0 + 128, :])
            nc.scalar.dma_start(wt[(0, 0)][:, 0:NK * 128],
                                wall[r0:r0 + 128, 0:NK * 128])
            nc.scalar.dma_start(wt[(0, 0)][:, NK * 128:3 * NK * 128],
                                wall[r0:r0 + 128, NK * 128:3 * NK * 128])
            nc.gpsimd.dma_start(t0_sb[:], t0n[:])
            nc.gpsimd.dma_start(cw_sb[:], cwt[:])
            vs_dma(nc.scalar, 1, 0)
            vs_dma(nc.sync, 2, 0)
            vs_dma(nc.scalar, 0, 1)
            vs_dma(nc.sync, 1, 1)
            vs_dma(nc.scalar, 2, 1)
            nc.sync.dma_start(zgt[:], zgm[:])
            nc.scalar.dma_start(wm_sb[:], wm[:])

            # ---- HAM warm-up (writes psum_o; the first real projection
            # matmul clears it via start=True) ----
            psum_o = opool.tile([O, B], F32)
            for _ in range(DUMMY_MMS):
                nc.tensor.matmul(psum_o[:], dz[:, 0:O], dz[:], start=True,
                                 stop=True)
            n_proj = 0
            pending = []   # deferred projection matmuls (src, unit, rows)

            def flush_proj():
                nonlocal n_proj
                for src_t, unit, rows in pending:
                    n_proj += 1
                    nc.tensor.matmul(psum_o[:],
                                     cw_sb[0:rows, unit * O:(unit + 1) * O],
                                     src_t[:],
                                     start=(n_proj == 1),
                                     stop=(n_proj == N_UNITS))
                pending.clear()

            def epilogue(li, p, xs, unit, rows):
                """Per-layer recurrence for one k-tile unit on ACT + DVE."""
                if li == 0:
                    x1 = xpool.tile([rows, B], F32, tag="x1")
                    nc.scalar.copy(x1[:], p[:])
                    xs["x1"] = x1
                elif li == 1:
                    m2 = tpool.tile([rows, B], F32, tag="m2")
                    x2 = xpool.tile([rows, B], F32, tag="x2")
                    nc.vector.tensor_mul(m2[:], p[:], xs["x1"][:])
                    nc.scalar.add(x2[:], m2[:], t0_sb[0:rows, unit:unit + 1])
                    xs["x2"] = x2
                else:
                    u = tpool.tile([rows, B], F32, tag="u")
                    x3 = xpool.tile([rows, B], F16, tag="x3")
                    nc.vector.tensor_mul(u[:], p[:], xs["x2"][:])
                    nc.gpsimd.tensor_sub(x3[:], u[:], xs["x1"][:])
                    pending.append((x3, unit, rows))

            # A-phase then B-phase, slots in order, 3 layers each
            for t in range(2):
                for s in range(FULL_SLOTS):
                    w, zs = wt[(s, t)], zt[(s, t)]
                    units = {}
                    for li in range(3):
                        flush_proj()
                        p = ppool.tile([128, B], F32, tag="ps")
                        for k in range(NK):
                            nc.tensor.matmul(
                                p[:], w[:, (li * NK + k) * 128:
                                        (li * NK + k + 1) * 128],
                                zs[:, k * B:(k + 1) * B],
                                start=(k == 0), stop=(k == NK - 1))
                        epilogue(li, p, units, 2 * s + t, 128)

            # mini unit last: its short [32, B] recurrence minimizes the tail
            mini = {}
            for li in range(3):
                p = ppool.tile([MINI_COLS, B], F32, tag="ps")
                flush_proj()
                for k in range(NKM):
                    lhsT = wm_sb[:, (li * NKM + k) * MINI_COLS:
                                 (li * NKM + k + 1) * MINI_COLS]
                    nc.tensor.matmul(p[:], lhsT, zgt[:, k * B:(k + 1) * B],
                                     start=(k == 0), stop=(k == NKM - 1))
                epilogue(li, p, mini, 6, MINI_COLS)
            flush_proj()

            out_sb = cpool.tile([O, B], F32, tag="out")
            nc.vector.tensor_copy(out_sb[:], psum_o[:])
            nc.sync.dma_start(out[:], out_sb[:])

    nc.compile()
    return nc


_NC = None


def _get_nc():
    global _NC
    if _NC is None:
        _NC = _build_nc()
    return _NC


def _prepare_in_maps(z, T1, T2, T3, T0, C_w, mask):
    z = np.ascontiguousarray(np.asarray(z, dtype=np.float32).reshape(B, D2))
    T1 = np.asarray(T1, dtype=np.float32)
    T2 = np.asarray(T2, dtype=np.float32)
    T3 = np.asarray(T3, dtype=np.float32)
    T0 = np.asarray(T0, dtype=np.float32)
    C_w = np.asarray(C_w, dtype=np.float32)
    mask = np.asarray(mask, dtype=np.float32)

    zT = np.ascontiguousarray(z.T)                   # [4096, 256]
    Ts = (T1, T2, T3)
    scales = (1.0, 2.0, 2.0)
    g24_cols = _group_cols_jmajor(G_SPLIT)
    g24_win = np.arange(128 * G_SPLIT, 128 * G_SPLIT + 1024)

    # concat row r (0..639) -> (chunk r//WC, window index r%WC)
    r = np.arange(8 * WC)
    cidx, widx = r // WC, r % WC
    win_rows = [128 * cidx + _WIN[t][widx] for t in range(2)]   # into 1024-win

    in_maps = []
    for c in range(N_CORES):
        i0 = 3 * c

        zst_h = np.zeros((6, 128, NK * B), np.float32)
        wall_h = np.zeros((6, 128, 3, NK, 128), np.float32)
        for s in range(FULL_SLOTS):
            g = i0 + s
            cols = _group_cols_jmajor(g)
            colsets = (cols[0:125], cols[125:250])
            zwin = zT[128 * g:128 * g + 1024]                   # [1024, B]
            for t in range(2):
                rows = win_rows[t]
                zc = zwin[rows]                                 # [640, B]
                zst_h[t * 3 + s] = (zc.reshape(NK, 128, B)
                                    .transpose(1, 0, 2).reshape(128, NK * B))
                ix = np.ix_(colsets[t], np.arange(128 * g, 128 * g + 1024))
                for li, (T, sc) in enumerate(zip(Ts, scales)):
                    AT = (sc * T[ix] * mask[ix]).T              # [1024, 125]
                    wc = AT[rows]                               # [640, 125]
                    wall_h[t * 3 + s, :, li, :, 0:125] = \
                        wc.reshape(NK, 128, 125).transpose(1, 0, 2)
        m = {"zst": np.ascontiguousarray(
                 zst_h.reshape(6 * 128, NK * B)).astype(np.float16),
             "wall": np.ascontiguousarray(
                 wall_h.reshape(6 * 128, 3 * NK * 128)).astype(np.float16)}

        # mini unit: j-sorted slice of group 24, host-shifted WM-row window
        nmini = _MINI_N[c]
        o = _MINI_OFF[c]
        mcols = g24_cols[o:o + nmini]
        jmin = int(o // 10)
        ccols = 2 * jmin + np.arange(WM // 2)
        valid = ccols < 64
        ccols = np.where(valid, ccols, 0)
        win_m = np.array([(w % 2) * 64 + ccols[w // 2] for w in range(WM)],
                         dtype=np.int64)
        vmask = np.repeat(valid, 2).astype(np.float32)[
            np.arange(WM) // 2 * 2 + np.arange(WM) % 2]
        vmask = np.array([valid[w // 2] for w in range(WM)], np.float32)
        rm = np.arange(8 * WM)
        rows_m = 128 * (rm // WM) + win_m[rm % WM]

        zgwin = zT[128 * G_SPLIT:128 * G_SPLIT + 1024]          # [1024, B]
        zc = zgwin[rows_m] * np.tile(vmask, 8)[:, None]         # [384, B]
        m["zgm"] = np.ascontiguousarray(
            zc.reshape(NKM, 128, B).transpose(1, 0, 2)
            .reshape(128, NKM * B)).astype(np.float16)

        wmh = np.zeros((128, 3, NKM, MINI_COLS), np.float32)
        for li, (T, sc) in enumerate(zip(Ts, scales)):
            A = (sc * T[np.ix_(mcols, g24_win)] * mask[np.ix_(mcols, g24_win)]).T
            wc = A[rows_m] * np.tile(vmask, 8)[:, None]         # [384, nmini]
            wmh[:, li, :, 0:nmini] = \
                wc.reshape(NKM, 128, nmini).transpose(1, 0, 2)
        m["wm"] = np.ascontiguousarray(
            wmh.reshape(128, 3 * NKM * MINI_COLS)).astype(np.float16)

        # t0 / C_w per unit: units 0..5 = full slots (2s+t), 6 = mini
        t0nh = np.zeros((128, 8), np.float32)
        cwth = np.zeros((128, N_UNITS * O), np.float32)
        for s in range(FULL_SLOTS):
            cols = _group_cols_jmajor(i0 + s)
            t0nh[0:125, 2 * s] = -T0[cols[0:125]]
            t0nh[0:125, 2 * s + 1] = -T0[cols[125:250]]
            cwth[0:125, (2 * s) * O:(2 * s + 1) * O] = C_w[:, cols[0:125]].T
            cwth[0:125, (2 * s + 1) * O:(2 * s + 2) * O] = C_w[:, cols[125:250]].T
        t0nh[0:nmini, 6] = -T0[mcols]
        cwth[0:nmini, 6 * O:7 * O] = C_w[:, mcols].T
        m["t0n"] = t0nh
        m["cwt"] = cwth.astype(np.float16)
        in_maps.append(m)
    return in_maps


def kernel(z, T1, T2, T3, T0, C_w, C_b, mask):
    nc = _get_nc()
    in_maps = _prepare_in_maps(z, T1, T2, T3, T0, C_w, mask)
    res = run_bass_kernel_spmd(nc, in_maps, core_ids=list(range(N_CORES)))
    total = np.zeros((O, B), np.float32)
    for c in range(N_CORES):
        total += res.results[c]["out"]
    C_b = np.asarray(C_b, dtype=np.float32)
    return (total.T + C_b).astype(np.float32)


# revision 25
# speedup vs baseline: 1.0985x; 1.0985x over previous
"""Trainium2 Bass kernel for the stacked-Chebyshev locally-connected net.

Reference computation (B=256, k=6250, d*d=4096, O=10):
    x1 = z @ (mask*T1).T
    x2 = 2*(z @ (mask*T2).T)*x1 - T0
    x3 = 2*(z @ (mask*T3).T)*x2 - x1
    out = x3 @ C_w.T + C_b

The mask is a locally-connected conv pattern: 16x16 patch, stride 2, 25x25
positions, stacked 10x.  Rows sharing patch-row index i have a contiguous
1024-wide support in d (8 chunks of 2 image rows); grouping by i cuts the
contraction from 4096 to 1024.  Sorting each group's 250 k-columns by
patch-column j and splitting 125/125 narrows the support inside every
chunk to 40 of 64 image columns = 80 of 128 rows (ascending column
interleave for the low-j "A" tile, descending for the high-j "B" tile).
The 8 per-chunk 80-row windows are CONCATENATED into one 640-row
contraction stream = exactly 5 full K=128 matmuls instead of 8 — 37.5%
fewer PE cycles and 37.5% less weight DMA than the dense form, with all
transfers spanning 128 partitions (full SDMA port utilization) and all
matmuls full-K (the HAM clock-gate only stays at 2.4 GHz under full-row
activity).  The price: the z stream becomes per-(slot, tile) instead of
shared, which is small next to the weight savings.

Sharding: 25 i-groups over 8 cores; every core gets 3 whole groups plus
1/8 of group 24 (a 32-column "mini" unit, j-sorted per core so its
support fits a host-shifted 48-row window; 8 x 48 = 384 = 3 matmuls).

Performance structure:
  - the two HWDGE rings carry (z, w) of each virtual slot on opposite
    rings, in exact PE consumption order; ~16 large DMAs total.
  - dummy matmuls on a memset tile run during the DMA preamble so the
    PE's HAM clock-gate is warm when the real matmuls start.
  - all matmuls are fp16 (fast-weight-load stays on, LDWEIGHTS hides).
  - the Chebyshev recurrence is split across the scalar (ACT) and vector
    (DVE) engines; per-core partial outputs are summed on the host.
"""

import numpy as np

import concourse.bass as bass
import concourse.mybir as mybir
import concourse.tile as tile
from concourse import bacc
from concourse.bass_utils import run_bass_kernel_spmd

F32 = mybir.dt.float32
F16 = mybir.dt.float16

B = 256          # batch
O = 10           # output classes
D2 = 4096        # d*d
N_CORES = 8
FULL_SLOTS = 3   # whole groups per core
WC = 80          # window rows per chunk (40 image cols x 2 rows)
NK = 5           # K=128 matmuls per (slot, tile, layer): 8*WC/128
WM = 48          # mini window rows per chunk
NKM = 3          # K=128 matmuls per mini layer: 8*WM/128
MINI_COLS = 32   # k-columns of the shared group-24 mini unit (<=32 real)
G_SPLIT = 24     # the group split across all 8 cores
N_UNITS = 2 * FULL_SLOTS + 1
DUMMY_MMS = 14   # HAM warm-up matmuls during the DMA preamble

# columns of group G_SPLIT owned by each core (6x31 + 2x32 = 250)
_MINI_N = (31, 31, 31, 31, 31, 31, 32, 32)
_MINI_OFF = tuple(int(x) for x in np.cumsum((0,) + _MINI_N[:-1]))


def _group_cols_jmajor(i):
    """k-column indices of group i, j-major (j outer, stack inner)."""
    return np.array(
        [s * 625 + i * 25 + j for j in range(25) for s in range(10)], dtype=np.int64
    )


# window pixel indices within a 128-px chunk (2 image rows of 64):
# ascending layout index w <-> pixel (w%2)*64 + w//2     (cols 0..39)
# descending layout index w <-> pixel (w%2)*64 + 63-w//2 (cols 63..24)
_WIN = (np.array([(w % 2) * 64 + w // 2 for w in range(WC)], dtype=np.int64),
        np.array([(w % 2) * 64 + 63 - w // 2 for w in range(WC)], dtype=np.int64))


def _build_nc():
    nc = bacc.Bacc(
        "TRN2", target_bir_lowering=False, debug=False, num_devices=N_CORES
    )
    # z contraction streams: row block (t*3+s) holds the 640-row concat
    # chunk-blocked as [128, NK*B]
    zst = nc.dram_tensor("zst", [6 * 128, NK * B], F16, kind="ExternalInput").ap()
    # weights: row block (t*3+s), cols (li*NK + k)*128 + m
    wall = nc.dram_tensor("wall", [6 * 128, 3 * NK * 128], F16,
                          kind="ExternalInput").ap()
    # mini z stream and weights
    zgm = nc.dram_tensor("zgm", [128, NKM * B], F16, kind="ExternalInput").ap()
    wm = nc.dram_tensor("wm", [128, 3 * NKM * MINI_COLS], F16,
                        kind="ExternalInput").ap()
    # negated T0 (additive bias on the scalar engine); col = unit index
    t0n = nc.dram_tensor("t0n", [128, 8], F32, kind="ExternalInput").ap()
    cwt = nc.dram_tensor("cwt", [128, N_UNITS * O], F16, kind="ExternalInput").ap()
    out = nc.dram_tensor("out", [O, B], F32, kind="ExternalOutput").ap()

    with tile.TileContext(nc) as tc:
        with (
            tc.tile_pool(name="zpool", bufs=1) as zpool,
            tc.tile_pool(name="cpool", bufs=1) as cpool,
            tc.tile_pool(name="wpool", bufs=1) as wpool,
            tc.tile_pool(name="xpool", bufs=6) as xpool,
            tc.tile_pool(name="tpool", bufs=4) as tpool,
            tc.tile_pool(name="ppool", bufs=7, space="PSUM") as ppool,
            tc.tile_pool(name="opool", bufs=1, space="PSUM") as opool,
        ):
            dz = zpool.tile([128, B], F16, tag="dz")
            nc.vector.memset(dz[:], 0.0)

            zt, wt = {}, {}
            for t in range(2):
                for s in range(FULL_SLOTS):
                    ztile = zpool.tile([128, NK * B], F16, tag=f"z{s}{t}",
                                       name=f"z{s}{t}")
                    wtile = wpool.tile([128, 3 * NK * 128], F16,
                                       tag=f"w{s}{t}", name=f"w{s}{t}")
                    zt[(s, t)] = ztile
                    wt[(s, t)] = wtile
            zgt = cpool.tile([128, NKM * B], F16, tag="zg")
            wm_sb = cpool.tile([128, 3 * NKM * MINI_COLS], F16, tag="wm")
            t0_sb = cpool.tile([128, 8], F32, tag="t0")
            cw_sb = cpool.tile([128, N_UNITS * O], F16, tag="cw")

            def vs_dma(zeng, s, t):
                r = (t * 3 + s) * 128
                zeng.dma_start(zt[(s, t)][:], zst[r:r + 128, :])
                weng = nc.scalar if zeng is nc.sync else nc.sync
                weng.dma_start(wt[(s, t)][:], wall[r:r + 128, :])

            # ---- DMA issue plan: (z, w) of each vslot on opposite rings,
            # in consumption order; the very first vslot's weights arrive
            # layer-1 first so matmuls can start early ----
            r0 = 0
            nc.sync.dma_start(zt[(0, 0)][:], zst[r0:r0 + 128, :])
            nc.scalar.dma_start(wt[(0, 0)][:, 0:NK * 128],
                                wall[r0:r0 + 128, 0:NK * 128])
            nc.scalar.dma_start(wt[(0, 0)][:, NK * 128:3 * NK * 128],
                                wall[r0:r0 + 128, NK * 128:3 * NK * 128])
            nc.gpsimd.dma_start(t0_sb[:], t0n[:])
            nc.gpsimd.dma_start(cw_sb[:], cwt[:])
            vs_dma(nc.scalar, 1, 0)
            vs_dma(nc.sync, 2, 0)
            vs_dma(nc.scalar, 0, 1)
            vs_dma(nc.sync, 1, 1)
            vs_dma(nc.scalar, 2, 1)
            nc.sync.dma_start(zgt[:], zgm[:])
            nc.scalar.dma_start(wm_sb[:], wm[:])

            # ---- HAM warm-up (writes psum_o; the first real projection
            # matmul clears it via start=True) ----
            psum_o = opool.tile([O, B], F32)
            for _ in range(DUMMY_MMS):
                nc.tensor.matmul(psum_o[:], dz[:, 0:O], dz[:], start=True,
                                 stop=True)
            n_proj = 0
            pending = []   # deferred projection matmuls (src, unit, rows)

            def flush_proj():
                nonlocal n_proj
                for src_t, unit, rows in pending:
                    n_proj += 1
                    nc.tensor.matmul(psum_o[:],
                                     cw_sb[0:rows, unit * O:(unit + 1) * O],
                                     src_t[:],
                                     start=(n_proj == 1),
                                     stop=(n_proj == N_UNITS))
                pending.clear()

            def epilogue(li, p, xs, unit, rows):
                """Per-layer recurrence for one k-tile unit on ACT + DVE."""
                if li == 0:
                    x1 = xpool.tile([rows, B], F32, tag="x1")
                    nc.scalar.copy(x1[:], p[:])
                    xs["x1"] = x1
                elif li == 1:
                    m2 = tpool.tile([rows, B], F32, tag="m2")
                    x2 = xpool.tile([rows, B], F32, tag="x2")
                    nc.vector.tensor_mul(m2[:], p[:], xs["x1"][:])
                    nc.scalar.add(x2[:], m2[:], t0_sb[0:rows, unit:unit + 1])
                    xs["x2"] = x2
                else:
                    u = tpool.tile([rows, B], F32, tag="u")
                    x3 = xpool.tile([rows, B], F16, tag="x3")
                    nc.vector.tensor_mul(u[:], p[:], xs["x2"][:])
                    nc.vector.tensor_sub(x3[:], u[:], xs["x1"][:])
                    pending.append((x3, unit, rows))

            # A-phase then B-phase, slots in order, 3 layers each
            for t in range(2):
                for s in range(FULL_SLOTS):
                    w, zs = wt[(s, t)], zt[(s, t)]
                    units = {}
                    for li in range(3):
                        flush_proj()
                        p = ppool.tile([128, B], F32, tag="ps")
                        for k in range(NK):
                            nc.tensor.matmul(
                                p[:], w[:, (li * NK + k) * 128:
                                        (li * NK + k + 1) * 128],
                                zs[:, k * B:(k + 1) * B],
                                start=(k == 0), stop=(k == NK - 1))
                        epilogue(li, p, units, 2 * s + t, 128)

            # mini unit last: its short [32, B] recurrence minimizes the tail
            mini = {}
            for li in range(3):
                p = ppool.tile([MINI_COLS, B], F32, tag="ps")
                flush_proj()
                for k in range(NKM):
                    lhsT = wm_sb[:, (li * NKM + k) * MINI_COLS:
                                 (li * NKM + k + 1) * MINI_COLS]
                    nc.tensor.matmul(p[:], lhsT, zgt[:, k * B:(k + 1) * B],
                                     start=(k == 0), stop=(k == NKM - 1))
                epilogue(li, p, mini, 6, MINI_COLS)
            flush_proj()

            out_sb = cpool.tile([O, B], F32, tag="out")
            nc.vector.tensor_copy(out_sb[:], psum_o[:])
            nc.sync.dma_start(out[:], out_sb[:])

    nc.compile()
    return nc


_NC = None


def _get_nc():
    global _NC
    if _NC is None:
        _NC = _build_nc()
    return _NC


def _prepare_in_maps(z, T1, T2, T3, T0, C_w, mask):
    z = np.ascontiguousarray(np.asarray(z, dtype=np.float32).reshape(B, D2))
    T1 = np.asarray(T1, dtype=np.float32)
    T2 = np.asarray(T2, dtype=np.float32)
    T3 = np.asarray(T3, dtype=np.float32)
    T0 = np.asarray(T0, dtype=np.float32)
    C_w = np.asarray(C_w, dtype=np.float32)
    mask = np.asarray(mask, dtype=np.float32)

    zT = np.ascontiguousarray(z.T)                   # [4096, 256]
    Ts = (T1, T2, T3)
    scales = (1.0, 2.0, 2.0)
    g24_cols = _group_cols_jmajor(G_SPLIT)
    g24_win = np.arange(128 * G_SPLIT, 128 * G_SPLIT + 1024)

    # concat row r (0..639) -> (chunk r//WC, window index r%WC)
    r = np.arange(8 * WC)
    cidx, widx = r // WC, r % WC
    win_rows = [128 * cidx + _WIN[t][widx] for t in range(2)]   # into 1024-win

    in_maps = []
    for c in range(N_CORES):
        i0 = 3 * c

        zst_h = np.zeros((6, 128, NK * B), np.float32)
        wall_h = np.zeros((6, 128, 3, NK, 128), np.float32)
        for s in range(FULL_SLOTS):
            g = i0 + s
            cols = _group_cols_jmajor(g)
            colsets = (cols[0:125], cols[125:250])
            zwin = zT[128 * g:128 * g + 1024]                   # [1024, B]
            for t in range(2):
                rows = win_rows[t]
                zc = zwin[rows]                                 # [640, B]
                zst_h[t * 3 + s] = (zc.reshape(NK, 128, B)
                                    .transpose(1, 0, 2).reshape(128, NK * B))
                ix = np.ix_(colsets[t], np.arange(128 * g, 128 * g + 1024))
                for li, (T, sc) in enumerate(zip(Ts, scales)):
                    AT = (sc * T[ix] * mask[ix]).T              # [1024, 125]
                    wc = AT[rows]                               # [640, 125]
                    wall_h[t * 3 + s, :, li, :, 0:125] = \
                        wc.reshape(NK, 128, 125).transpose(1, 0, 2)
        m = {"zst": np.ascontiguousarray(
                 zst_h.reshape(6 * 128, NK * B)).astype(np.float16),
             "wall": np.ascontiguousarray(
                 wall_h.reshape(6 * 128, 3 * NK * 128)).astype(np.float16)}

        # mini unit: j-sorted slice of group 24, host-shifted WM-row window
        nmini = _MINI_N[c]
        o = _MINI_OFF[c]
        mcols = g24_cols[o:o + nmini]
        jmin = int(o // 10)
        ccols = 2 * jmin + np.arange(WM // 2)
        valid = ccols < 64
        ccols = np.where(valid, ccols, 0)
        win_m = np.array([(w % 2) * 64 + ccols[w // 2] for w in range(WM)],
                         dtype=np.int64)
        vmask = np.repeat(valid, 2).astype(np.float32)[
            np.arange(WM) // 2 * 2 + np.arange(WM) % 2]
        vmask = np.array([valid[w // 2] for w in range(WM)], np.float32)
        rm = np.arange(8 * WM)
        rows_m = 128 * (rm // WM) + win_m[rm % WM]

        zgwin = zT[128 * G_SPLIT:128 * G_SPLIT + 1024]          # [1024, B]
        zc = zgwin[rows_m] * np.tile(vmask, 8)[:, None]         # [384, B]
        m["zgm"] = np.ascontiguousarray(
            zc.reshape(NKM, 128, B).transpose(1, 0, 2)
            .reshape(128, NKM * B)).astype(np.float16)

        wmh = np.zeros((128, 3, NKM, MINI_COLS), np.float32)
        for li, (T, sc) in enumerate(zip(Ts, scales)):
            A = (sc * T[np.ix_(mcols, g24_win)] * mask[np.ix_(mcols, g24_win)]).T
            wc = A[rows_m] * np.tile(vmask, 8)[:, None]         # [384, nmini]
            wmh[:, li, :, 0:nmini] = \
                wc.reshape(NKM, 128, nmini).transpose(1, 0, 2)
        m["wm"] = np.ascontiguousarray(
            wmh.reshape(128, 3 * NKM * MINI_COLS)).astype(np.float16)

        # t0 / C_w per unit: units 0..5 = full slots (2s+t), 6 = mini
        t0nh = np.zeros((128, 8), np.float32)
        cwth = np.zeros((128, N_UNITS * O), np.float32)
        for s in range(FULL_SLOTS):
            cols = _group_cols_jmajor(i0 + s)
            t0nh[0:125, 2 * s] = -T0[cols[0:125]]
            t0nh[0:125, 2 * s + 1] = -T0[cols[125:250]]
            cwth[0:125, (2 * s) * O:(2 * s + 1) * O] = C_w[:, cols[0:125]].T
            cwth[0:125, (2 * s + 1) * O:(2 * s + 2) * O] = C_w[:, cols[125:250]].T
        t0nh[0:nmini, 6] = -T0[mcols]
        cwth[0:nmini, 6 * O:7 * O] = C_w[:, mcols].T
        m["t0n"] = t0nh
        m["cwt"] = cwth.astype(np.float16)
        in_maps.append(m)
    return in_maps


def kernel(z, T1, T2, T3, T0, C_w, C_b, mask):
    nc = _get_nc()
    in_maps = _prepare_in_maps(z, T1, T2, T3, T0, C_w, mask)
    res = run_bass_kernel_spmd(nc, in_maps, core_ids=list(range(N_CORES)))
    total = np.zeros((O, B), np.float32)
    for c in range(N_CORES):
        total += res.results[c]["out"]
    C_b = np.asarray(C_b, dtype=np.float32)
    return (total.T + C_b).astype(np.float32)


# revision 26
# speedup vs baseline: 1.1285x; 1.0273x over previous
"""Trainium2 Bass kernel for the stacked-Chebyshev locally-connected net.

Reference computation (B=256, k=6250, d*d=4096, O=10):
    x1 = z @ (mask*T1).T
    x2 = 2*(z @ (mask*T2).T)*x1 - T0
    x3 = 2*(z @ (mask*T3).T)*x2 - x1
    out = x3 @ C_w.T + C_b

The mask is a locally-connected conv pattern: 16x16 patch, stride 2, 25x25
positions, stacked 10x.  Rows sharing patch-row index i have a contiguous
1024-wide support in d (8 chunks of 2 image rows); grouping by i cuts the
contraction from 4096 to 1024.  Sorting each group's 250 k-columns by
patch-column j and splitting 125/125 narrows the support inside every
chunk to 40 of 64 image columns = 80 of 128 rows (ascending column
interleave for the low-j "A" tile, descending for the high-j "B" tile).
The 8 per-chunk 80-row windows are CONCATENATED into one 640-row
contraction stream = exactly 5 full K=128 matmuls instead of 8 — 37.5%
fewer PE cycles and 37.5% less weight DMA than the dense form, with all
transfers spanning 128 partitions (full SDMA port utilization) and all
matmuls full-K (the HAM clock-gate only stays at 2.4 GHz under full-row
activity).  The price: the z stream becomes per-(slot, tile) instead of
shared, which is small next to the weight savings.

Sharding: 25 i-groups over 8 cores; every core gets 3 whole groups plus
1/8 of group 24 (a 32-column "mini" unit, j-sorted per core so its
support fits a host-shifted 48-row window; 8 x 48 = 384 = 3 matmuls).

Performance structure:
  - the two HWDGE rings carry (z, w) of each virtual slot on opposite
    rings, in exact PE consumption order; ~16 large DMAs total.
  - dummy matmuls on a memset tile run during the DMA preamble so the
    PE's HAM clock-gate is warm when the real matmuls start.
  - all matmuls are fp16 (fast-weight-load stays on, LDWEIGHTS hides).
  - the Chebyshev recurrence is split across the scalar (ACT) and vector
    (DVE) engines; per-core partial outputs are summed on the host.
"""

import numpy as np

import concourse.bass as bass
import concourse.mybir as mybir
import concourse.tile as tile
from concourse import bacc
from concourse.bass_utils import run_bass_kernel_spmd

F32 = mybir.dt.float32
F16 = mybir.dt.float16

B = 256          # batch
O = 10           # output classes
D2 = 4096        # d*d
N_CORES = 8
FULL_SLOTS = 3   # whole groups per core
WC = 80          # window rows per chunk (40 image cols x 2 rows)
NK = 5           # K=128 matmuls per (slot, tile, layer): 8*WC/128
WM = 48          # mini window rows per chunk
NKM = 3          # K=128 matmuls per mini layer: 8*WM/128
MINI_COLS = 32   # k-columns of the shared group-24 mini unit (<=32 real)
G_SPLIT = 24     # the group split across all 8 cores
N_UNITS = 2 * FULL_SLOTS + 1
DUMMY_MMS = 16   # HAM warm-up matmuls during the DMA preamble

# columns of group G_SPLIT owned by each core (6x31 + 2x32 = 250)
_MINI_N = (31, 31, 31, 31, 31, 31, 32, 32)
_MINI_OFF = tuple(int(x) for x in np.cumsum((0,) + _MINI_N[:-1]))


def _group_cols_jmajor(i):
    """k-column indices of group i, j-major (j outer, stack inner)."""
    return np.array(
        [s * 625 + i * 25 + j for j in range(25) for s in range(10)], dtype=np.int64
    )


# window pixel indices within a 128-px chunk (2 image rows of 64):
# ascending layout index w <-> pixel (w%2)*64 + w//2     (cols 0..39)
# descending layout index w <-> pixel (w%2)*64 + 63-w//2 (cols 63..24)
_WIN = (np.array([(w % 2) * 64 + w // 2 for w in range(WC)], dtype=np.int64),
        np.array([(w % 2) * 64 + 63 - w // 2 for w in range(WC)], dtype=np.int64))


def _build_nc():
    nc = bacc.Bacc(
        "TRN2", target_bir_lowering=False, debug=False, num_devices=N_CORES
    )
    # z contraction streams: row block (t*3+s) holds the 640-row concat
    # chunk-blocked as [128, NK*B]
    zst = nc.dram_tensor("zst", [6 * 128, NK * B], F16, kind="ExternalInput").ap()
    # weights: row block (t*3+s), cols (li*NK + k)*128 + m
    wall = nc.dram_tensor("wall", [6 * 128, 3 * NK * 128], F16,
                          kind="ExternalInput").ap()
    # mini z stream and weights
    zgm = nc.dram_tensor("zgm", [128, NKM * B], F16, kind="ExternalInput").ap()
    wm = nc.dram_tensor("wm", [128, 3 * NKM * MINI_COLS], F16,
                        kind="ExternalInput").ap()
    # negated T0 (additive bias on the scalar engine); col = unit index
    t0n = nc.dram_tensor("t0n", [128, 8], F32, kind="ExternalInput").ap()
    cwt = nc.dram_tensor("cwt", [128, N_UNITS * O], F16, kind="ExternalInput").ap()
    out = nc.dram_tensor("out", [O, B], F32, kind="ExternalOutput").ap()

    with tile.TileContext(nc) as tc:
        with (
            tc.tile_pool(name="zpool", bufs=1) as zpool,
            tc.tile_pool(name="cpool", bufs=1) as cpool,
            tc.tile_pool(name="wpool", bufs=1) as wpool,
            tc.tile_pool(name="xpool", bufs=6) as xpool,
            tc.tile_pool(name="tpool", bufs=4) as tpool,
            tc.tile_pool(name="ppool", bufs=7, space="PSUM") as ppool,
            tc.tile_pool(name="opool", bufs=1, space="PSUM") as opool,
        ):
            dz = zpool.tile([128, B], F16, tag="dz")
            nc.vector.memset(dz[:], 0.0)

            zt, wt = {}, {}
            for t in range(2):
                for s in range(FULL_SLOTS):
                    ztile = zpool.tile([128, NK * B], F16, tag=f"z{s}{t}",
                                       name=f"z{s}{t}")
                    wtile = wpool.tile([128, 3 * NK * 128], F16,
                                       tag=f"w{s}{t}", name=f"w{s}{t}")
                    zt[(s, t)] = ztile
                    wt[(s, t)] = wtile
            zgt = cpool.tile([128, NKM * B], F16, tag="zg")
            wm_sb = cpool.tile([128, 3 * NKM * MINI_COLS], F16, tag="wm")
            t0_sb = cpool.tile([128, 8], F32, tag="t0")
            cw_sb = cpool.tile([128, N_UNITS * O], F16, tag="cw")

            def vs_dma(zeng, s, t):
                r = (t * 3 + s) * 128
                zeng.dma_start(zt[(s, t)][:], zst[r:r + 128, :])
                weng = nc.scalar if zeng is nc.sync else nc.sync
                weng.dma_start(wt[(s, t)][:], wall[r:r + 128, :])

            # ---- DMA issue plan: (z, w) of each vslot on opposite rings,
            # in consumption order ----
            vs_dma(nc.sync, 0, 0)
            nc.gpsimd.dma_start(t0_sb[:], t0n[:])
            nc.gpsimd.dma_start(cw_sb[:], cwt[:])
            vs_dma(nc.scalar, 1, 0)
            vs_dma(nc.sync, 2, 0)
            vs_dma(nc.scalar, 0, 1)
            vs_dma(nc.sync, 1, 1)
            vs_dma(nc.scalar, 2, 1)
            nc.sync.dma_start(zgt[:], zgm[:])
            nc.scalar.dma_start(wm_sb[:], wm[:])

            # ---- HAM warm-up (writes psum_o; the first real projection
            # matmul clears it via start=True) ----
            psum_o = opool.tile([O, B], F32)
            for _ in range(DUMMY_MMS):
                nc.tensor.matmul(psum_o[:], dz[:, 0:O], dz[:], start=True,
                                 stop=True)
            n_proj = 0
            pending = []   # deferred projection matmuls (src, unit, rows)

            def flush_proj():
                nonlocal n_proj
                for src_t, unit, rows in pending:
                    n_proj += 1
                    nc.tensor.matmul(psum_o[:],
                                     cw_sb[0:rows, unit * O:(unit + 1) * O],
                                     src_t[:],
                                     start=(n_proj == 1),
                                     stop=(n_proj == N_UNITS))
                pending.clear()

            def epilogue(li, p, xs, unit, rows):
                """Per-layer recurrence for one k-tile unit on ACT + DVE."""
                if li == 0:
                    x1 = xpool.tile([rows, B], F32, tag="x1")
                    nc.scalar.copy(x1[:], p[:])
                    xs["x1"] = x1
                elif li == 1:
                    m2 = tpool.tile([rows, B], F32, tag="m2")
                    x2 = xpool.tile([rows, B], F32, tag="x2")
                    nc.vector.tensor_mul(m2[:], p[:], xs["x1"][:])
                    nc.scalar.add(x2[:], m2[:], t0_sb[0:rows, unit:unit + 1])
                    xs["x2"] = x2
                else:
                    u = tpool.tile([rows, B], F32, tag="u")
                    x3 = xpool.tile([rows, B], F16, tag="x3")
                    nc.vector.tensor_mul(u[:], p[:], xs["x2"][:])
                    nc.vector.tensor_sub(x3[:], u[:], xs["x1"][:])
                    pending.append((x3, unit, rows))

            # A-phase then B-phase, slots in order, 3 layers each
            for t in range(2):
                for s in range(FULL_SLOTS):
                    w, zs = wt[(s, t)], zt[(s, t)]
                    units = {}
                    for li in range(3):
                        flush_proj()
                        p = ppool.tile([128, B], F32, tag="ps")
                        for k in range(NK):
                            nc.tensor.matmul(
                                p[:], w[:, (li * NK + k) * 128:
                                        (li * NK + k + 1) * 128],
                                zs[:, k * B:(k + 1) * B],
                                start=(k == 0), stop=(k == NK - 1))
                        epilogue(li, p, units, 2 * s + t, 128)

            # mini unit last: its short [32, B] recurrence minimizes the tail
            mini = {}
            for li in range(3):
                p = ppool.tile([MINI_COLS, B], F32, tag="ps")
                flush_proj()
                for k in range(NKM):
                    lhsT = wm_sb[:, (li * NKM + k) * MINI_COLS:
                                 (li * NKM + k + 1) * MINI_COLS]
                    nc.tensor.matmul(p[:], lhsT, zgt[:, k * B:(k + 1) * B],
                                     start=(k == 0), stop=(k == NKM - 1))
                epilogue(li, p, mini, 6, MINI_COLS)
            flush_proj()

            out_sb = cpool.tile([O, B], F32, tag="out")
            nc.vector.tensor_copy(out_sb[:], psum_o[:])
            nc.sync.dma_start(out[:], out_sb[:])

    nc.compile()
    return nc


_NC = None


def _get_nc():
    global _NC
    if _NC is None:
        _NC = _build_nc()
    return _NC


def _prepare_in_maps(z, T1, T2, T3, T0, C_w, mask):
    z = np.ascontiguousarray(np.asarray(z, dtype=np.float32).reshape(B, D2))
    T1 = np.asarray(T1, dtype=np.float32)
    T2 = np.asarray(T2, dtype=np.float32)
    T3 = np.asarray(T3, dtype=np.float32)
    T0 = np.asarray(T0, dtype=np.float32)
    C_w = np.asarray(C_w, dtype=np.float32)
    mask = np.asarray(mask, dtype=np.float32)

    zT = np.ascontiguousarray(z.T)                   # [4096, 256]
    Ts = (T1, T2, T3)
    scales = (1.0, 2.0, 2.0)
    g24_cols = _group_cols_jmajor(G_SPLIT)
    g24_win = np.arange(128 * G_SPLIT, 128 * G_SPLIT + 1024)

    # concat row r (0..639) -> (chunk r//WC, window index r%WC)
    r = np.arange(8 * WC)
    cidx, widx = r // WC, r % WC
    win_rows = [128 * cidx + _WIN[t][widx] for t in range(2)]   # into 1024-win

    in_maps = []
    for c in range(N_CORES):
        i0 = 3 * c

        zst_h = np.zeros((6, 128, NK * B), np.float32)
        wall_h = np.zeros((6, 128, 3, NK, 128), np.float32)
        for s in range(FULL_SLOTS):
            g = i0 + s
            cols = _group_cols_jmajor(g)
            colsets = (cols[0:125], cols[125:250])
            zwin = zT[128 * g:128 * g + 1024]                   # [1024, B]
            for t in range(2):
                rows = win_rows[t]
                zc = zwin[rows]                                 # [640, B]
                zst_h[t * 3 + s] = (zc.reshape(NK, 128, B)
                                    .transpose(1, 0, 2).reshape(128, NK * B))
                ix = np.ix_(colsets[t], np.arange(128 * g, 128 * g + 1024))
                for li, (T, sc) in enumerate(zip(Ts, scales)):
                    AT = (sc * T[ix] * mask[ix]).T              # [1024, 125]
                    wc = AT[rows]                               # [640, 125]
                    wall_h[t * 3 + s, :, li, :, 0:125] = \
                        wc.reshape(NK, 128, 125).transpose(1, 0, 2)
        m = {"zst": np.ascontiguousarray(
                 zst_h.reshape(6 * 128, NK * B)).astype(np.float16),
             "wall": np.ascontiguousarray(
                 wall_h.reshape(6 * 128, 3 * NK * 128)).astype(np.float16)}

        # mini unit: j-sorted slice of group 24, host-shifted WM-row window
        nmini = _MINI_N[c]
        o = _MINI_OFF[c]
        mcols = g24_cols[o:o + nmini]
        jmin = int(o // 10)
        ccols = 2 * jmin + np.arange(WM // 2)
        valid = ccols < 64
        ccols = np.where(valid, ccols, 0)
        win_m = np.array([(w % 2) * 64 + ccols[w // 2] for w in range(WM)],
                         dtype=np.int64)
        vmask = np.repeat(valid, 2).astype(np.float32)[
            np.arange(WM) // 2 * 2 + np.arange(WM) % 2]
        vmask = np.array([valid[w // 2] for w in range(WM)], np.float32)
        rm = np.arange(8 * WM)
        rows_m = 128 * (rm // WM) + win_m[rm % WM]

        zgwin = zT[128 * G_SPLIT:128 * G_SPLIT + 1024]          # [1024, B]
        zc = zgwin[rows_m] * np.tile(vmask, 8)[:, None]         # [384, B]
        m["zgm"] = np.ascontiguousarray(
            zc.reshape(NKM, 128, B).transpose(1, 0, 2)
            .reshape(128, NKM * B)).astype(np.float16)

        wmh = np.zeros((128, 3, NKM, MINI_COLS), np.float32)
        for li, (T, sc) in enumerate(zip(Ts, scales)):
            A = (sc * T[np.ix_(mcols, g24_win)] * mask[np.ix_(mcols, g24_win)]).T
            wc = A[rows_m] * np.tile(vmask, 8)[:, None]         # [384, nmini]
            wmh[:, li, :, 0:nmini] = \
                wc.reshape(NKM, 128, nmini).transpose(1, 0, 2)
        m["wm"] = np.ascontiguousarray(
            wmh.reshape(128, 3 * NKM * MINI_COLS)).astype(np.float16)

        # t0 / C_w per unit: units 0..5 = full slots (2s+t), 6 = mini
        t0nh = np.zeros((128, 8), np.float32)
        cwth = np.zeros((128, N_UNITS * O), np.float32)
        for s in range(FULL_SLOTS):
            cols = _group_cols_jmajor(i0 + s)
            t0nh[0:125, 2 * s] = -T0[cols[0:125]]
            t0nh[0:125, 2 * s + 1] = -T0[cols[125:250]]
            cwth[0:125, (2 * s) * O:(2 * s + 1) * O] = C_w[:, cols[0:125]].T
            cwth[0:125, (2 * s + 1) * O:(2 * s + 2) * O] = C_w[:, cols[125:250]].T
        t0nh[0:nmini, 6] = -T0[mcols]
        cwth[0:nmini, 6 * O:7 * O] = C_w[:, mcols].T
        m["t0n"] = t0nh
        m["cwt"] = cwth.astype(np.float16)
        in_maps.append(m)
    return in_maps


def kernel(z, T1, T2, T3, T0, C_w, C_b, mask):
    nc = _get_nc()
    in_maps = _prepare_in_maps(z, T1, T2, T3, T0, C_w, mask)
    res = run_bass_kernel_spmd(nc, in_maps, core_ids=list(range(N_CORES)))
    total = np.zeros((O, B), np.float32)
    for c in range(N_CORES):
        total += res.results[c]["out"]
    C_b = np.asarray(C_b, dtype=np.float32)
    return (total.T + C_b).astype(np.float32)


# revision 27
# speedup vs baseline: 1.1473x; 1.0167x over previous
"""Trainium2 Bass kernel for the stacked-Chebyshev locally-connected net.

Reference computation (B=256, k=6250, d*d=4096, O=10):
    x1 = z @ (mask*T1).T
    x2 = 2*(z @ (mask*T2).T)*x1 - T0
    x3 = 2*(z @ (mask*T3).T)*x2 - x1
    out = x3 @ C_w.T + C_b

The mask is a locally-connected conv pattern: 16x16 patch, stride 2, 25x25
positions, stacked 10x.  Rows sharing patch-row index i have a contiguous
1024-wide support in d (8 chunks of 2 image rows); grouping by i cuts the
contraction from 4096 to 1024.  Sorting each group's 250 k-columns by
patch-column j and splitting 125/125 narrows the support inside every
chunk to 40 of 64 image columns = 80 of 128 rows (ascending column
interleave for the low-j "A" tile, descending for the high-j "B" tile).
The 8 per-chunk 80-row windows are CONCATENATED into one 640-row
contraction stream = exactly 5 full K=128 matmuls instead of 8 — 37.5%
fewer PE cycles and 37.5% less weight DMA than the dense form, with all
transfers spanning 128 partitions (full SDMA port utilization) and all
matmuls full-K (the HAM clock-gate only stays at 2.4 GHz under full-row
activity).  The price: the z stream becomes per-(slot, tile) instead of
shared, which is small next to the weight savings.

Sharding: 25 i-groups over 8 cores; every core gets 3 whole groups plus
1/8 of group 24 (a 32-column "mini" unit, j-sorted per core so its
support fits a host-shifted 48-row window; 8 x 48 = 384 = 3 matmuls).

Performance structure:
  - the two HWDGE rings carry (z, w) of each virtual slot on opposite
    rings, in exact PE consumption order; ~16 large DMAs total.
  - dummy matmuls on a memset tile run during the DMA preamble so the
    PE's HAM clock-gate is warm when the real matmuls start.
  - all matmuls are fp16 (fast-weight-load stays on, LDWEIGHTS hides).
  - the Chebyshev recurrence is split across the scalar (ACT) and vector
    (DVE) engines; per-core partial outputs are summed on the host.
"""

import numpy as np

import concourse.bass as bass
import concourse.mybir as mybir
import concourse.tile as tile
from concourse import bacc
from concourse.bass_utils import run_bass_kernel_spmd

F32 = mybir.dt.float32
F16 = mybir.dt.float16

B = 256          # batch
O = 10           # output classes
D2 = 4096        # d*d
N_CORES = 8
FULL_SLOTS = 3   # whole groups per core
WC = 80          # window rows per chunk (40 image cols x 2 rows)
NK = 5           # K=128 matmuls per (slot, tile, layer): 8*WC/128
WM = 48          # mini window rows per chunk
NKM = 3          # K=128 matmuls per mini layer: 8*WM/128
MINI_COLS = 32   # k-columns of the shared group-24 mini unit (<=32 real)
G_SPLIT = 24     # the group split across all 8 cores
N_UNITS = 2 * FULL_SLOTS + 1
DUMMY_MMS = 16   # HAM warm-up matmuls during the DMA preamble

# columns of group G_SPLIT owned by each core (6x31 + 2x32 = 250)
_MINI_N = (31, 31, 31, 31, 31, 31, 32, 32)
_MINI_OFF = tuple(int(x) for x in np.cumsum((0,) + _MINI_N[:-1]))


def _group_cols_jmajor(i):
    """k-column indices of group i, j-major (j outer, stack inner)."""
    return np.array(
        [s * 625 + i * 25 + j for j in range(25) for s in range(10)], dtype=np.int64
    )


# window pixel indices within a 128-px chunk (2 image rows of 64):
# ascending layout index w <-> pixel (w%2)*64 + w//2     (cols 0..39)
# descending layout index w <-> pixel (w%2)*64 + 63-w//2 (cols 63..24)
_WIN = (np.array([(w % 2) * 64 + w // 2 for w in range(WC)], dtype=np.int64),
        np.array([(w % 2) * 64 + 63 - w // 2 for w in range(WC)], dtype=np.int64))


def _build_nc():
    nc = bacc.Bacc(
        "TRN2", target_bir_lowering=False, debug=False, num_devices=N_CORES
    )
    # z contraction streams: row block (t*3+s) holds the 640-row concat
    # chunk-blocked as [128, NK*B]
    zst = nc.dram_tensor("zst", [6 * 128, NK * B], F16, kind="ExternalInput").ap()
    # weights: row block (t*3+s), cols (li*NK + k)*128 + m
    wall = nc.dram_tensor("wall", [6 * 128, 3 * NK * 128], F16,
                          kind="ExternalInput").ap()
    # mini z stream and weights
    zgm = nc.dram_tensor("zgm", [128, NKM * B], F16, kind="ExternalInput").ap()
    wm = nc.dram_tensor("wm", [128, 3 * NKM * MINI_COLS], F16,
                        kind="ExternalInput").ap()
    # negated T0 (additive bias on the scalar engine); col = unit index
    t0n = nc.dram_tensor("t0n", [128, 8], F32, kind="ExternalInput").ap()
    cwt = nc.dram_tensor("cwt", [128, N_UNITS * O], F16, kind="ExternalInput").ap()
    out = nc.dram_tensor("out", [O, B], F32, kind="ExternalOutput").ap()

    with tile.TileContext(nc) as tc:
        with (
            tc.tile_pool(name="zpool", bufs=1) as zpool,
            tc.tile_pool(name="cpool", bufs=1) as cpool,
            tc.tile_pool(name="wpool", bufs=1) as wpool,
            tc.tile_pool(name="xpool", bufs=6) as xpool,
            tc.tile_pool(name="tpool", bufs=4) as tpool,
            tc.tile_pool(name="ppool", bufs=7, space="PSUM") as ppool,
            tc.tile_pool(name="opool", bufs=1, space="PSUM") as opool,
        ):
            dz = zpool.tile([128, B], F16, tag="dz")
            nc.vector.memset(dz[:], 0.0)

            zt, wt = {}, {}
            for t in range(2):
                for s in range(FULL_SLOTS):
                    ztile = zpool.tile([128, NK * B], F16, tag=f"z{s}{t}",
                                       name=f"z{s}{t}")
                    wtile = wpool.tile([128, 3 * NK * 128], F16,
                                       tag=f"w{s}{t}", name=f"w{s}{t}")
                    zt[(s, t)] = ztile
                    wt[(s, t)] = wtile
            zgt = cpool.tile([128, NKM * B], F16, tag="zg")
            wm_sb = cpool.tile([128, 3 * NKM * MINI_COLS], F16, tag="wm")
            t0_sb = cpool.tile([128, 8], F32, tag="t0")
            cw_sb = cpool.tile([128, N_UNITS * O], F16, tag="cw")

            def vs_dma(zeng, s, t):
                r = (t * 3 + s) * 128
                zeng.dma_start(zt[(s, t)][:], zst[r:r + 128, :])
                weng = nc.scalar if zeng is nc.sync else nc.sync
                weng.dma_start(wt[(s, t)][:], wall[r:r + 128, :])

            # ---- DMA issue plan: (z, w) of each vslot on opposite rings,
            # in consumption order ----
            vs_dma(nc.sync, 0, 0)
            nc.gpsimd.dma_start(t0_sb[:], t0n[:])
            nc.gpsimd.dma_start(cw_sb[:], cwt[:])
            vs_dma(nc.scalar, 1, 0)
            vs_dma(nc.sync, 2, 0)
            vs_dma(nc.scalar, 0, 1)
            nc.sync.dma_start(zgt[:], zgm[:])
            nc.scalar.dma_start(wm_sb[:], wm[:])
            vs_dma(nc.sync, 1, 1)
            vs_dma(nc.scalar, 2, 1)

            # ---- HAM warm-up (writes psum_o; the first real projection
            # matmul clears it via start=True) ----
            psum_o = opool.tile([O, B], F32)
            for _ in range(DUMMY_MMS):
                nc.tensor.matmul(psum_o[:], dz[:, 0:O], dz[:], start=True,
                                 stop=True)
            n_proj = 0
            pending = []   # deferred projection matmuls (src, unit, rows)

            def flush_proj():
                nonlocal n_proj
                for src_t, unit, rows in pending:
                    n_proj += 1
                    nc.tensor.matmul(psum_o[:],
                                     cw_sb[0:rows, unit * O:(unit + 1) * O],
                                     src_t[:],
                                     start=(n_proj == 1),
                                     stop=(n_proj == N_UNITS))
                pending.clear()

            def epilogue(li, p, xs, unit, rows):
                """Per-layer recurrence for one k-tile unit on ACT + DVE."""
                if li == 0:
                    x1 = xpool.tile([rows, B], F32, tag="x1")
                    nc.scalar.copy(x1[:], p[:])
                    xs["x1"] = x1
                elif li == 1:
                    m2 = tpool.tile([rows, B], F32, tag="m2")
                    x2 = xpool.tile([rows, B], F32, tag="x2")
                    nc.vector.tensor_mul(m2[:], p[:], xs["x1"][:])
                    nc.scalar.add(x2[:], m2[:], t0_sb[0:rows, unit:unit + 1])
                    xs["x2"] = x2
                else:
                    u = tpool.tile([rows, B], F32, tag="u")
                    x3 = xpool.tile([rows, B], F16, tag="x3")
                    nc.vector.tensor_mul(u[:], p[:], xs["x2"][:])
                    nc.vector.tensor_sub(x3[:], u[:], xs["x1"][:])
                    pending.append((x3, unit, rows))

            def full_vslot(s, t):
                w, zs = wt[(s, t)], zt[(s, t)]
                units = {}
                for li in range(3):
                    flush_proj()
                    p = ppool.tile([128, B], F32, tag="ps")
                    for k in range(NK):
                        nc.tensor.matmul(
                            p[:], w[:, (li * NK + k) * 128:
                                    (li * NK + k + 1) * 128],
                            zs[:, k * B:(k + 1) * B],
                            start=(k == 0), stop=(k == NK - 1))
                    epilogue(li, p, units, 2 * s + t, 128)

            def mini_unit():
                mini = {}
                for li in range(3):
                    p = ppool.tile([MINI_COLS, B], F32, tag="ps")
                    flush_proj()
                    for k in range(NKM):
                        lhsT = wm_sb[:, (li * NKM + k) * MINI_COLS:
                                     (li * NKM + k + 1) * MINI_COLS]
                        nc.tensor.matmul(p[:], lhsT,
                                         zgt[:, k * B:(k + 1) * B],
                                         start=(k == 0),
                                         stop=(k == NKM - 1))
                    epilogue(li, p, mini, 6, MINI_COLS)

            # A-phase, then B0, mini mid-stream, B1, B2 — only one epilogue
            # chain (B2's) remains serialized at the very end
            full_vslot(0, 0)
            full_vslot(1, 0)
            full_vslot(2, 0)
            full_vslot(0, 1)
            mini_unit()
            full_vslot(1, 1)
            full_vslot(2, 1)
            flush_proj()

            out_sb = cpool.tile([O, B], F32, tag="out")
            nc.vector.tensor_copy(out_sb[:], psum_o[:])
            nc.sync.dma_start(out[:], out_sb[:])

    nc.compile()
    return nc


_NC = None


def _get_nc():
    global _NC
    if _NC is None:
        _NC = _build_nc()
    return _NC


def _prepare_in_maps(z, T1, T2, T3, T0, C_w, mask):
    z = np.ascontiguousarray(np.asarray(z, dtype=np.float32).reshape(B, D2))
    T1 = np.asarray(T1, dtype=np.float32)
    T2 = np.asarray(T2, dtype=np.float32)
    T3 = np.asarray(T3, dtype=np.float32)
    T0 = np.asarray(T0, dtype=np.float32)
    C_w = np.asarray(C_w, dtype=np.float32)
    mask = np.asarray(mask, dtype=np.float32)

    zT = np.ascontiguousarray(z.T)                   # [4096, 256]
    Ts = (T1, T2, T3)
    scales = (1.0, 2.0, 2.0)
    g24_cols = _group_cols_jmajor(G_SPLIT)
    g24_win = np.arange(128 * G_SPLIT, 128 * G_SPLIT + 1024)

    # concat row r (0..639) -> (chunk r//WC, window index r%WC)
    r = np.arange(8 * WC)
    cidx, widx = r // WC, r % WC
    win_rows = [128 * cidx + _WIN[t][widx] for t in range(2)]   # into 1024-win

    in_maps = []
    for c in range(N_CORES):
        i0 = 3 * c

        zst_h = np.zeros((6, 128, NK * B), np.float32)
        wall_h = np.zeros((6, 128, 3, NK, 128), np.float32)
        for s in range(FULL_SLOTS):
            g = i0 + s
            cols = _group_cols_jmajor(g)
            colsets = (cols[0:125], cols[125:250])
            zwin = zT[128 * g:128 * g + 1024]                   # [1024, B]
            for t in range(2):
                rows = win_rows[t]
                zc = zwin[rows]                                 # [640, B]
                zst_h[t * 3 + s] = (zc.reshape(NK, 128, B)
                                    .transpose(1, 0, 2).reshape(128, NK * B))
                ix = np.ix_(colsets[t], np.arange(128 * g, 128 * g + 1024))
                for li, (T, sc) in enumerate(zip(Ts, scales)):
                    AT = (sc * T[ix] * mask[ix]).T              # [1024, 125]
                    wc = AT[rows]                               # [640, 125]
                    wall_h[t * 3 + s, :, li, :, 0:125] = \
                        wc.reshape(NK, 128, 125).transpose(1, 0, 2)
        m = {"zst": np.ascontiguousarray(
                 zst_h.reshape(6 * 128, NK * B)).astype(np.float16),
             "wall": np.ascontiguousarray(
                 wall_h.reshape(6 * 128, 3 * NK * 128)).astype(np.float16)}

        # mini unit: j-sorted slice of group 24, host-shifted WM-row window
        nmini = _MINI_N[c]
        o = _MINI_OFF[c]
        mcols = g24_cols[o:o + nmini]
        jmin = int(o // 10)
        ccols = 2 * jmin + np.arange(WM // 2)
        valid = ccols < 64
        ccols = np.where(valid, ccols, 0)
        win_m = np.array([(w % 2) * 64 + ccols[w // 2] for w in range(WM)],
                         dtype=np.int64)
        vmask = np.repeat(valid, 2).astype(np.float32)[
            np.arange(WM) // 2 * 2 + np.arange(WM) % 2]
        vmask = np.array([valid[w // 2] for w in range(WM)], np.float32)
        rm = np.arange(8 * WM)
        rows_m = 128 * (rm // WM) + win_m[rm % WM]

        zgwin = zT[128 * G_SPLIT:128 * G_SPLIT + 1024]          # [1024, B]
        zc = zgwin[rows_m] * np.tile(vmask, 8)[:, None]         # [384, B]
        m["zgm"] = np.ascontiguousarray(
            zc.reshape(NKM, 128, B).transpose(1, 0, 2)
            .reshape(128, NKM * B)).astype(np.float16)

        wmh = np.zeros((128, 3, NKM, MINI_COLS), np.float32)
        for li, (T, sc) in enumerate(zip(Ts, scales)):
            A = (sc * T[np.ix_(mcols, g24_win)] * mask[np.ix_(mcols, g24_win)]).T
            wc = A[rows_m] * np.tile(vmask, 8)[:, None]         # [384, nmini]
            wmh[:, li, :, 0:nmini] = \
                wc.reshape(NKM, 128, nmini).transpose(1, 0, 2)
        m["wm"] = np.ascontiguousarray(
            wmh.reshape(128, 3 * NKM * MINI_COLS)).astype(np.float16)

        # t0 / C_w per unit: units 0..5 = full slots (2s+t), 6 = mini
        t0nh = np.zeros((128, 8), np.float32)
        cwth = np.zeros((128, N_UNITS * O), np.float32)
        for s in range(FULL_SLOTS):
            cols = _group_cols_jmajor(i0 + s)
            t0nh[0:125, 2 * s] = -T0[cols[0:125]]
            t0nh[0:125, 2 * s + 1] = -T0[cols[125:250]]
            cwth[0:125, (2 * s) * O:(2 * s + 1) * O] = C_w[:, cols[0:125]].T
            cwth[0:125, (2 * s + 1) * O:(2 * s + 2) * O] = C_w[:, cols[125:250]].T
        t0nh[0:nmini, 6] = -T0[mcols]
        cwth[0:nmini, 6 * O:7 * O] = C_w[:, mcols].T
        m["t0n"] = t0nh
        m["cwt"] = cwth.astype(np.float16)
        in_maps.append(m)
    return in_maps


def kernel(z, T1, T2, T3, T0, C_w, C_b, mask):
    nc = _get_nc()
    in_maps = _prepare_in_maps(z, T1, T2, T3, T0, C_w, mask)
    res = run_bass_kernel_spmd(nc, in_maps, core_ids=list(range(N_CORES)))
    total = np.zeros((O, B), np.float32)
    for c in range(N_CORES):
        total += res.results[c]["out"]
    C_b = np.asarray(C_b, dtype=np.float32)
    return (total.T + C_b).astype(np.float32)
